# revision 19
# baseline (speedup 1.0000x reference)
"""Multi-headed self-attention (B=8, S=1024, D=768, H=12) on 8 TRN2 cores.

Sharding: data-parallel over batch -- core i computes batch element i.
Per-core kernel (all operands pre-transposed + bf16-cast on host):
    Qt = (Wq @ x.T + bq)      [D, S]   (o on partitions), bf16
    Kt = (Wk @ x.T + bk)      [D, S]   bf16
    V  = (x @ Wv.T + bv)      [S, D]   bf16, augmented with a ones column/head
    St_h = Kt_h^T-slices @ Qt_h   -> scores transposed [k, q] (psum fp32)
    Et = exp(St/8 + maskbias[k])  (ACT, mask bias per-partition) -> bf16
    PVt'_h = V'_h.T @ Et_h        [65, q] fp32; row 64 = sum_k Et = Z[q]
    out_h.T = PVt'_h[0:64] / Z    -> outT rows h*64..h*64+63 (fp32)
Host transposes outT back.

Schedule: flat software pipeline over 96 (head, kc) units with skew 2.
Per unit the PE stream is [proj-microtask, scores(i+2), pv(i)]: the Q/K
projection work for the NEXT oc block is spread 3 matmuls at a time so
the PE never runs dry while exp(i) (the ACT long pole per unit) is
still in flight. Softmax normalization stays on-chip: reciprocal of
the Z row (DVE) + partition_broadcast (Pool) + multiply (DVE); no
DRAM round trip.
"""

import numpy as np
import ml_dtypes

import concourse.bacc as bacc
import concourse.tile as tile
from concourse import mybir
from concourse.bass_utils import run_bass_kernel_spmd

B, S, D, H = 8, 1024, 768, 12
HD = D // H  # 64
N_CORES = 8
SC = S // 128  # 8 key/seq chunks
OC = D // 128  # 6 output chunks (2 heads each)
DC = D // 128  # 6 contraction chunks
NT = 512  # matmul moving-dim tile
QT = S // NT  # 2
F32 = mybir.dt.float32
F32R = mybir.dt.float32r
BF16 = mybir.dt.bfloat16
F16 = mybir.dt.float16

HW = HD + 1  # per-head V width incl. ones column


def build():
    nc = bacc.Bacc("TRN2", target_bir_lowering=False, debug=False, num_devices=N_CORES)
    xT = nc.dram_tensor("xT", [D, S], BF16, kind="ExternalInput").ap()
    wqT = nc.dram_tensor("wqT", [D, D], BF16, kind="ExternalInput").ap()
    wkT = nc.dram_tensor("wkT", [D, D], BF16, kind="ExternalInput").ap()
    wvT = nc.dram_tensor("wvT", [D, D], BF16, kind="ExternalInput").ap()
    bq = nc.dram_tensor("bq", [D], F32, kind="ExternalInput").ap()
    bk = nc.dram_tensor("bk", [D], F32, kind="ExternalInput").ap()
    bvb = nc.dram_tensor("bvb", [128, D], F32, kind="ExternalInput").ap()
    mb = nc.dram_tensor("mb", [S], F32, kind="ExternalInput").ap()
    outT = nc.dram_tensor("outT", [D, S], F16, kind="ExternalOutput").ap()

    with tile.TileContext(nc) as tc:
        with (
            tc.tile_pool(name="const", bufs=1) as const,
            tc.tile_pool(name="qk", bufs=2) as qk_pool,
            tc.tile_pool(name="et", bufs=6) as et_pool,
            tc.tile_pool(name="epi", bufs=2) as epi_pool,
            tc.tile_pool(name="st", bufs=3, space="PSUM") as st_ps,
            tc.tile_pool(name="pv", bufs=2, space="PSUM") as pv_ps,
            tc.tile_pool(name="dram", bufs=2, space="DRAM") as dram_pool,
        ):
            # ---------- constant / weight loads ----------
            # priority order: x and Wv feed the V projection that starts the
            # PE; Wq/Wk are only needed once qk_proj(0) begins.
            xt = [const.tile([128, S], BF16, tag=f"xt{c}", name=f"xt{c}") for c in range(DC)]
            wq = [const.tile([128, D], BF16, tag=f"wq{c}", name=f"wq{c}") for c in range(DC)]
            wk = [const.tile([128, D], BF16, tag=f"wk{c}", name=f"wk{c}") for c in range(DC)]
            wv = [const.tile([128, D], BF16, tag=f"wv{c}", name=f"wv{c}") for c in range(DC)]
            # big loads on the sync DGE: x+Wv interleaved first (the V
            # projection blocks on them), then Wq/Wk. One whole-tile call
            # each -- every dma_start costs ~0.6us of serial sequencer
            # issue time, so fewer+bigger wins at startup.
            for c in range(DC):
                nc.sync.dma_start(xt[c][:], xT[c * 128:(c + 1) * 128, :])
                nc.sync.dma_start(wv[c][:], wvT[c * 128:(c + 1) * 128, :])
            for c in range(DC):
                nc.sync.dma_start(wq[c][:], wqT[c * 128:(c + 1) * 128, :])
                nc.sync.dma_start(wk[c][:], wkT[c * 128:(c + 1) * 128, :])

            # small operands via the otherwise-idle gpsimd DGE so they are
            # neither queued behind the big loads nor eating sync-sequencer
            # issue slots (the first exp needs mb_t, the first bias-adds
            # need bq/bk/bvb)
            bq_t = const.tile([128, OC], F32, tag="bq")
            nc.gpsimd.dma_start(bq_t[:], bq.rearrange("(c p) -> p c", p=128))
            bk_t = const.tile([128, OC], F32, tag="bk")
            nc.gpsimd.dma_start(bk_t[:], bk.rearrange("(c p) -> p c", p=128))
            mb_t = const.tile([128, SC], F32, tag="mb")
            nc.gpsimd.dma_start(mb_t[:], mb.rearrange("(c p) -> p c", p=128))
            bvb_t = const.tile([128, D], F32, tag="bvb")
            nc.gpsimd.dma_start(bvb_t[:], bvb[:])
            # tiny dummy exp on a constant tile pulls the ~2.7us ACT table
            # load off the critical path without a DMA dependency
            warm = const.tile([128, 1], F32, tag="warm")
            nc.vector.memset(warm[:], 0.0)
            nc.scalar.activation(
                warm[:], warm[:], mybir.ActivationFunctionType.Exp
            )
            # ones row for the last-head PE-based 1/Z partition broadcast
            ones_row = const.tile([1, HD], F32R, tag="ones_row")
            nc.vector.memset(ones_row[:].bitcast(F32), 1.0)

            # ---------- V projection -> vaug [sc][128, H*65] ----------
            vaug = [const.tile([128, H * HW], BF16, tag=f"va{sc}", name=f"va{sc}") for sc in range(SC)]
            for sc in range(SC):
                ones_cols = vaug[sc][:].rearrange("p (h w) -> p h w", h=H)[:, :, HD:HW]
                nc.vector.memset(ones_cols, 1.0)

            def v_piece(sc):
                # both 512/256 halves per c-chunk back to back: the second
                # matmul reuses the stationary xt slice already in the PE
                vp = st_ps.tile([128, D], F32, tag="st", name=f"vp{sc}")
                for c in range(DC):
                    nc.tensor.matmul(
                        vp[:, 0:NT],
                        xt[c][:, sc * 128:(sc + 1) * 128],
                        wv[c][:, 0:NT],
                        start=(c == 0),
                        stop=(c == DC - 1),
                    )
                    nc.tensor.matmul(
                        vp[:, NT:D],
                        xt[c][:, sc * 128:(sc + 1) * 128],
                        wv[c][:, NT:D],
                        start=(c == 0),
                        stop=(c == DC - 1),
                    )
                nc.vector.tensor_add(
                    vaug[sc][:].rearrange("p (h w) -> p h w", h=H)[:, :, 0:HD],
                    vp[:].rearrange("p (h w) -> p h w", w=HD),
                    bvb_t[:].rearrange("p (h w) -> p h w", w=HD),
                )

            # ---------- Q/K projection, emitted in 3-matmul half-pieces ----
            wmap = {"q": (wq, bq_t), "k": (wk, bk_t)}

            def qk_alloc(oc):
                return {
                    name: qk_pool.tile([128, S], BF16, tag=name, name=f"{name}t{oc}")
                    for name in ("q", "k")
                }

            def qk_half(oc, dsts, name, qt, half, pstate):
                w_t, b_t = wmap[name]
                key = (oc, name, qt)
                if half == 0:
                    pstate[key] = st_ps.tile(
                        [128, NT], F32, tag="st", name=f"qkp{name}{qt}"
                    )
                p = pstate[key]
                c0 = half * (DC // 2)
                for c in range(c0, c0 + DC // 2):
                    nc.tensor.matmul(
                        p[:],
                        w_t[c][:, oc * 128:(oc + 1) * 128],
                        xt[c][:, qt * NT:(qt + 1) * NT],
                        start=(c == 0),
                        stop=(c == DC - 1),
                    )
                if half == 1:
                    nc.vector.tensor_scalar_add(
                        dsts[name][:, qt * NT:(qt + 1) * NT], p[:], b_t[:, oc:oc + 1]
                    )
                    del pstate[key]

            # ---------- attention: flat software pipeline, skew=2 ----------
            for sc in range(SC):
                v_piece(sc)
            qkts = {}
            pstate = {}
            qkts[0] = qk_alloc(0)
            for name in ("q", "k"):
                for qt in range(QT):
                    for half in (0, 1):
                        qk_half(0, qkts[0], name, qt, half, pstate)

            units = [(oc, hh, kc) for oc in range(OC) for hh in range(2)
                     for kc in range(SC)]
            NU = len(units)
            SKEW = 2
            st_tiles = {}
            pvq_map = {}

            # proj microtasks: for block oc, the 8 half-pieces of qk(oc+1)
            # fire on the odd units of block oc (subtile deps let the next
            # block's early scores run before the k-qt1 halves land)
            proj_tasks = {}
            for oc in range(OC - 1):
                seq = [(name, qt, half) for name in ("q", "k") for qt in range(QT)
                       for half in (0, 1)]
                for j, (name, qt, half) in enumerate(seq):
                    proj_tasks[16 * oc + 1 + 2 * j] = (oc + 1, name, qt, half)

            def emit_scores(i):
                oc, hh, kc = units[i]
                p0 = hh * 64
                qkt = qkts[oc]
                stt = st_ps.tile([128, S], F32, tag="st", name=f"st{i}")
                for qt in range(QT):
                    nc.tensor.matmul(
                        stt[:, qt * NT:(qt + 1) * NT],
                        qkt["k"][p0:p0 + 64, kc * 128:(kc + 1) * 128],
                        qkt["q"][p0:p0 + 64, qt * NT:(qt + 1) * NT],
                        tile_position=(p0, 0),
                    )
                st_tiles[i] = stt

            def emit_epilogue(oc, hh):
                gh = 2 * oc + hh
                last = gh == H - 1
                pvq = pvq_map.pop((oc, hh))
                pvs = epi_pool.tile([HW, S], F32, tag="pvs", name="pvs", bufs=3)
                zp = epi_pool.tile([128, SC], F32, tag="zp", name="zp", bufs=4)
                # Z row first so the reciprocal chain starts while the bulk
                # rows copy (DMA cannot read PSUM directly)
                for qt in range(QT):
                    nc.vector.tensor_copy(
                        pvs[HD:HW, qt * NT:(qt + 1) * NT], pvq[qt][HD:HW, :]
                    )
                nc.gpsimd.dma_start(
                    zp[:], pvs[HD:HW, :].rearrange("o (p c) -> o p c", c=SC)
                )
                for qt in range(QT):
                    nc.vector.tensor_copy(
                        pvs[0:HD, qt * NT:(qt + 1) * NT], pvq[qt][0:HD, :]
                    )
                nc.vector.reciprocal(zp[:], zp[:])
                oh = epi_pool.tile([HD, S], F16, tag="oh", name="oh", bufs=3)
                if last:
                    # exposed tail: skip one DMA hop by staging 1/Z as a
                    # [1, S] row in SBUF and broadcasting it across the 64
                    # partitions with a rank-1 ones matmul into PSUM (the
                    # st pool is drained by now, so the 2-bank tile is free)
                    rz1 = epi_pool.tile([1, S], F32R, tag="rz1", bufs=1)
                    nc.gpsimd.dma_start(
                        rz1[:].bitcast(F32).rearrange("o (p c) -> o p c", c=SC),
                        zp[:],
                    )
                    zbp = st_ps.tile([HD, S], F32, tag="st", name="zbp")
                    for qt in range(QT):
                        nc.tensor.matmul(
                            zbp[:, qt * NT:(qt + 1) * NT],
                            ones_row[:],
                            rz1[:, qt * NT:(qt + 1) * NT],
                        )
                    for qt in range(QT):
                        nc.vector.tensor_mul(
                            oh[:, qt * NT:(qt + 1) * NT],
                            pvs[0:HD, qt * NT:(qt + 1) * NT],
                            zbp[:, qt * NT:(qt + 1) * NT],
                        )
                        nc.sync.dma_start(
                            outT[gh * HD:(gh + 1) * HD, qt * NT:(qt + 1) * NT],
                            oh[:, qt * NT:(qt + 1) * NT],
                        )
                    return
                # steady state: bounce through DRAM for the
                # partition-broadcast read
                rzd = dram_pool.tile([S], F32, tag="rzd", name="rzd", bufs=4)
                nc.gpsimd.dma_start(rzd.rearrange("(p c) -> p c", c=SC), zp[:])
                zb = epi_pool.tile([HD, S], F32, tag="zb", name="zb", bufs=3)
                nc.gpsimd.dma_start(zb[:], rzd[:].partition_broadcast(HD))
                # multiply + store per qt half so the first out-DMA overlaps
                # the second multiply (fp16 out: host upcasts)
                for qt in range(QT):
                    nc.vector.tensor_mul(
                        oh[:, qt * NT:(qt + 1) * NT],
                        pvs[0:HD, qt * NT:(qt + 1) * NT],
                        zb[:, qt * NT:(qt + 1) * NT],
                    )
                    nc.sync.dma_start(
                        outT[gh * HD:(gh + 1) * HD, qt * NT:(qt + 1) * NT],
                        oh[:, qt * NT:(qt + 1) * NT],
                    )

            ett_map = {}

            def emit_pv(j):
                oc, hh, kc = units[j]
                ett = ett_map.pop(j)
                gh = 2 * oc + hh
                if kc == 0:
                    pvq_map[(oc, hh)] = [
                        pv_ps.tile([HW, NT], F32, tag="pv", name=f"pv{gh}_{qt}")
                        for qt in range(QT)
                    ]
                pvq = pvq_map[(oc, hh)]
                for qt in range(QT):
                    nc.tensor.matmul(
                        pvq[qt][:],
                        vaug[kc][:, gh * HW:(gh + 1) * HW],
                        ett[:, qt * NT:(qt + 1) * NT],
                        start=(kc == 0),
                        stop=(kc == SC - 1),
                    )
                if kc == SC - 1:
                    emit_epilogue(oc, hh)

            for i in range(SKEW):
                emit_scores(i)
            for i, (oc, hh, kc) in enumerate(units):
                task = proj_tasks.get(i)
                if task is not None:
                    toc, name, qt, half = task
                    if (name, qt, half) == ("q", 0, 0):
                        qkts[toc] = qk_alloc(toc)
                        qkts.pop(toc - 2, None)
                    qk_half(toc, qkts[toc], name, qt, half, pstate)
                if i + SKEW < NU:
                    emit_scores(i + SKEW)
                stt = st_tiles.pop(i)
                ett = et_pool.tile([128, S], BF16, tag="et", name=f"et{i}")
                nc.scalar.activation(
                    ett[:],
                    stt[:],
                    mybir.ActivationFunctionType.Exp,
                    bias=mb_t[:, kc:kc + 1],
                    scale=1.0 / np.sqrt(HD),
                )
                ett_map[i] = ett
                # pv deferred one unit: exp(i-1) has had a full unit of PE
                # work to complete, so pv(i-1) never stalls the array
                if i >= 1:
                    emit_pv(i - 1)
            emit_pv(NU - 1)

    nc.compile()
    return nc


_NC = None


def _get_nc():
    global _NC
    if _NC is None:
        _NC = build()
    return _NC


def _in_maps(x, mask, Wq, bq, Wk, bk, Wv, bv):
    x = np.asarray(x, dtype=np.float32)
    mask = np.asarray(mask)
    bf = ml_dtypes.bfloat16
    wqT = np.ascontiguousarray(np.asarray(Wq, dtype=np.float32).T.astype(bf))
    wkT = np.ascontiguousarray(np.asarray(Wk, dtype=np.float32).T.astype(bf))
    wvT = np.ascontiguousarray(np.asarray(Wv, dtype=np.float32).T.astype(bf))
    bq = np.asarray(bq, dtype=np.float32)
    bk = np.asarray(bk, dtype=np.float32)
    bvb = np.ascontiguousarray(
        np.broadcast_to(np.asarray(bv, dtype=np.float32), (128, D))
    )
    maps = []
    for c in range(N_CORES):
        maps.append(
            {
                "xT": np.ascontiguousarray(x[c].T.astype(bf)),
                "wqT": wqT,
                "wkT": wkT,
                "wvT": wvT,
                "bq": bq,
                "bk": bk,
                "bvb": bvb,
                "mb": (-10000.0 * (1.0 - mask[c].astype(np.float32))).astype(
                    np.float32
                ),
            }
        )
    return maps


def run(inputs, trace=False, **kw):
    nc = _get_nc()
    res = run_bass_kernel_spmd(
        nc, _in_maps(**inputs), list(range(N_CORES)), trace=trace, **kw
    )
    out = np.stack(
        [np.ascontiguousarray(res.results[c]["outT"].T) for c in range(N_CORES)]
    ).astype(np.float32)
    return out, res


def kernel(**inputs):
    out, _ = run(inputs)
    return out
